# revision 22
# baseline (speedup 1.0000x reference)
"""Trainium2 Bass kernel for CharPredictorMultirateFFN.

Model: emb = emb_table[tokens]; conv = relu(causal_conv1d(emb, K=16) + b);
logits = cat(emb, conv) @ lin_w.T + lin_b; out = softmax(logits).

Key algebraic restructure (tokens take only V=256 values):
  conv[s, h] = sum_d G_d[tok[s-d], h]   with G_d = U_{15-d},
  U[v,k,h] = sum_e emb[v,e] conv_w[h,e,k]
so the conv becomes 16 shifted one-hot matmuls with contract dim 256. The
one-hot is exact in fp8 and the tables are fp8 e4m3, so each tap is ONE
DoubleRow matmul (contract 256 in a single pass).

v2: polyphase KARATSUBA on the tap dimension. Split tokens into even/odd
phases (chi0[m]=tok[2m], chi1[m]=tok[2m+1]) and taps by delay parity
(A0_e = G_{2e}, A1_e = G_{2e+1}).  Then with half-rate 8-tap products
  P0 = A0 (*) chi0,  P1 = A1 (*) chi1,  Pm = (A0+A1) (*) (chi0+chi1)
the conv outputs are
  y_even[m] = P0[m] + P1[m-1]
  y_odd[m]  = Pm[m] - P0[m] - P1[m]
i.e. 24 half-rate tap-streams instead of 32 -> 25% less PE streaming time
(the kernel is purely PE-streaming-bound).  The recombination runs on the
otherwise-idle DVE/ACT engines, reading the product PSUM banks directly.
chi0+chi1 is a 2-hot vector (values {0,1,2}, exact in fp8).

Bias folds: conv_b into A0[e=0] (+b) and Am[e=0] (+b; the 2-hot hits the
fold twice, and -P0 removes one copy -> odd outputs get exactly +b).
lin_b is folded into the host-gathered emb-side logit rows.

Sharding: data-parallel over batch - 4 sequences per core on 8 cores, all
tables replicated, no collectives.
"""

import numpy as np
import ml_dtypes

B, S, V, E, H, K = 32, 2048, 256, 512, 1024, 16
NCORES = 8
SEQ_PER_CORE = B // NCORES            # 4
M = S // 2                            # 1024 half-positions per sequence
PADH = 8                              # left zero-pad (max tap back-reach)
MPAD = M + PADH + 8                   # 1040 (16-aligned)
NT = M // 512                         # 2 tiles of 512 half-positions per seq
H8 = H // 128                         # 8 h-chunks
F16 = np.float16
F8 = ml_dtypes.float8_e4m3fn

TRACE = False          # set True (e.g. from test.py) to capture NTFF profile
LAST_RESULT = None     # BassKernelResults of the most recent run

_NC_CACHE = {}


def _build_nc():
    """Build the Bass module (SPMD, identical program on every core)."""
    from contextlib import ExitStack
    import concourse.bacc as bacc
    import concourse.tile as tile
    import concourse.mybir as mybir

    f32 = mybir.dt.float32
    f16 = mybir.dt.float16
    f8 = mybir.dt.float8e4
    AF = mybir.ActivationFunctionType
    DR = mybir.MatmulPerfMode.DoubleRow

    nc = bacc.Bacc("TRN2", target_bir_lowering=False, debug=False,
                   num_devices=NCORES)

    # one-hot phase signals: [p, vh, phase(chi0,chi1,chim), seq, col]
    oh_d = nc.dram_tensor("oh", [128, 2, 3, SEQ_PER_CORE, MPAD], f8,
                          kind="ExternalInput").ap()
    # tables: [p, hc, prod(A0,A1,Am), tap, vh, c] - per-hc slabs contiguous
    ua_d = nc.dram_tensor("ua", [128, H8, 3, 8, 2, 128], f8,
                          kind="ExternalInput").ap()
    w2_d = nc.dram_tensor("w2", [128, H8, V], f16, kind="ExternalInput").ap()
    # host-gathered emb-half logit rows (lin_b folded in), phase-split:
    # [unit, p, mc, v] with unit = (b*NT + t)*2 + par, token halfpos
    # m = t*512 + mc*128 + p
    pe_d = nc.dram_tensor("pe", [SEQ_PER_CORE * NT * 2, 128, 4, V], f16,
                          kind="ExternalInput").ap()
    # out[halftok, par, v]; halftok = b*1024 + m; token = halftok*2+par
    out_d = nc.dram_tensor("out", [SEQ_PER_CORE * M, 2, V], f32,
                           kind="ExternalOutput").ap()

    with tile.TileContext(nc) as tc, ExitStack() as ctx:
        consts = ctx.enter_context(tc.tile_pool(name="consts", bufs=1))
        ua_t = consts.tile([128, H8, 3, 8, 2, 128], f8, name="ua_t")
        oh_t = consts.tile([128, 2, 3, SEQ_PER_CORE, MPAD], f8, name="oh_t")
        w2_t = consts.tile([128, H8, V], f16, name="w2_t")
        carry_t = consts.tile([128, H8, 1], f16, name="carry_t")

        # staggered loads ordered along the critical path: the first conv
        # unit consumes ua[:, hc=0, prod] in prod order against
        # oh[phase, seq=0, cols<528], so stream those first - per-prod
        # table slabs (256KB) on sync, per-phase oh heads (135KB) on the
        # Activation HWDGE queue - and backfill the rest afterwards.
        nc.scalar.dma_start(oh_t[:, :, 0, 0, 0:528], oh_d[:, :, 0, 0, 0:528])
        nc.sync.dma_start(ua_t[:, 0, 0], ua_d[:, 0, 0])
        nc.scalar.dma_start(oh_t[:, :, 1, 0, 0:528], oh_d[:, :, 1, 0, 0:528])
        nc.sync.dma_start(ua_t[:, 0, 1], ua_d[:, 0, 1])
        nc.scalar.dma_start(oh_t[:, :, 2, 0, 0:528], oh_d[:, :, 2, 0, 0:528])
        # Pm's head tables ride the (shorter) Activation queue so they don't
        # wait behind the P0/P1 slabs on sync
        nc.scalar.dma_start(ua_t[:, 0, 2], ua_d[:, 0, 2])
        nc.sync.dma_start(ua_t[:, 1], ua_d[:, 1])
        nc.scalar.dma_start(oh_t[:, :, :, 0, 528:MPAD],
                            oh_d[:, :, :, 0, 528:MPAD])
        for hc in range(2, H8):
            nc.sync.dma_start(ua_t[:, hc], ua_d[:, hc])
        for b in range(1, SEQ_PER_CORE):
            nc.scalar.dma_start(oh_t[:, :, :, b, :], oh_d[:, :, :, b, :])
        nc.sync.dma_start(w2_t[:], w2_d[:])

        pe_pool = ctx.enter_context(tc.tile_pool(name="pep", bufs=6))
        warm_pool = ctx.enter_context(tc.tile_pool(name="warm", bufs=1))
        rt_pool = ctx.enter_context(tc.tile_pool(name="rtp", bufs=2))
        s1_pool = ctx.enter_context(tc.tile_pool(name="s1p", bufs=4))
        t1_pool = ctx.enter_context(tc.tile_pool(name="t1p", bufs=4))
        cps = ctx.enter_context(tc.tile_pool(name="cps", bufs=6, space="PSUM"))
        lps = ctx.enter_context(tc.tile_pool(name="lps", bufs=2, space="PSUM"))
        sm_pool = ctx.enter_context(tc.tile_pool(name="smp", bufs=6))
        out_pool = ctx.enter_context(tc.tile_pool(name="outp", bufs=6))

        # PE warm-up: the HAM clock gate keeps the PE at 1.2 GHz until it
        # has seen ~3.4us of sustained activity.  The first real matmul
        # can't start until its tables land (~10us of DMA), so burn the
        # wait on dependency-free dummy matmuls over a memset scratch tile
        # and a throwaway PSUM bank - the real matmuls then start warm.
        wsc = warm_pool.tile([128, 2, 256], f8, name="wsc")
        nc.vector.memset(wsc[:], 0.0)
        wps = lps.tile([128, 256], f32, name="wps", tag="psl")
        for i in range(16):
            nc.tensor.matmul(wps[:], wsc[:, :, 0:128], wsc[:],
                             start=(i == 0), stop=(i == 15),
                             perf_mode=DR)

        def conv_emit(b, t, mid=None):
            """Karatsuba conv for 512 half-positions (1024 tokens).

            Returns rt tile [128, 2(par), H8, 512] f16 = relu(conv) with
            h on partitions, and the pe tiles for both parities.
            """
            pe_ts = []
            for par in range(2):
                pe_t = pe_pool.tile([128, 4, V], f16, name="pe_t", tag="pe")
                nc.sync.dma_start(pe_t[:], pe_d[(b * NT + t) * 2 + par])
                pe_ts.append(pe_t)
            rt = rt_pool.tile([128, 2, H8, 512], f16, name="rt", tag="rt")
            c0 = t * 512 + PADH
            for hc in range(H8):
                bp0 = cps.tile([128, 512], f32, name="bp0", tag="cp")
                bp1 = cps.tile([128, 512], f32, name="bp1", tag="cp")
                bpm = cps.tile([128, 512], f32, name="bpm", tag="cp")
                for prod, bank in ((0, bp0), (1, bp1), (2, bpm)):
                    for e in range(8):
                        nc.tensor.matmul(
                            bank[:],
                            ua_t[:, hc, prod, e],
                            oh_t[:, :, prod if prod < 2 else 2, b,
                                 c0 - e: c0 - e + 512],
                            start=(e == 0), stop=(e == 7),
                            perf_mode=DR)
                # S1[j] = P1[m0-1+j], j=0..512  (513 wide)
                s1 = s1_pool.tile([128, 513], f16, name="s1", tag="s1")
                if t == 0:
                    nc.vector.memset(s1[:, 0:1], 0.0)
                else:
                    nc.vector.tensor_copy(s1[:, 0:1], carry_t[:, hc, :])
                nc.scalar.copy(s1[:, 1:513], bp1[:])
                nc.scalar.copy(carry_t[:, hc, :], bp1[:, 511:512])
                # even: relu(P0[m] + P1[m-1])
                nc.vector.tensor_add(rt[:, 0, hc, :], bp0[:], s1[:, 0:512])
                nc.vector.tensor_scalar_max(rt[:, 0, hc, :],
                                            rt[:, 0, hc, :], 0.0)
                # odd: relu(Pm[m] - P1[m] - P0[m])
                t1 = t1_pool.tile([128, 512], f16, name="t1", tag="t1")
                nc.vector.tensor_sub(t1[:], bpm[:], s1[:, 1:513])
                nc.vector.tensor_sub(rt[:, 1, hc, :], t1[:], bp0[:])
                nc.vector.tensor_scalar_max(rt[:, 1, hc, :],
                                            rt[:, 1, hc, :], 0.0)
                if hc == 1 and mid is not None:
                    # interleave the previous tile's stage3 here: its PE
                    # matmuls need no conv PSUM banks, so they cover the
                    # tile-boundary window while DVE drains this tile's
                    # first combos and frees banks for hc2+
                    mid()
            return rt, pe_ts

        def stage3_emit(b, t, rt, pe_ts):
            """logits = rt@W2T (PE) + gathered emb rows (DVE), softmax."""
            for par in range(2):
                pe_t = pe_ts[par]
                for mc in range(4):
                    psl = lps.tile([128, V], f32, name="psl", tag="psl")
                    for h8 in range(H8):
                        nc.tensor.matmul(
                            psl[:], rt[:, par, h8, mc * 128:(mc + 1) * 128],
                            w2_t[:, h8, :],
                            start=(h8 == 0), stop=(h8 == H8 - 1))
                    li = sm_pool.tile([128, V], f32, name="li", tag="li")
                    nc.vector.tensor_add(li[:], psl[:], pe_t[:, mc, :])
                    et = sm_pool.tile([128, V], f32, name="et", tag="et")
                    ssum = sm_pool.tile([128, 1], f32, name="ssum", tag="ss")
                    nc.scalar.activation(et[:], li[:], AF.Exp,
                                         accum_out=ssum[:])
                    rec = sm_pool.tile([128, 1], f32, name="rec", tag="rec")
                    nc.vector.reciprocal(rec[:], ssum[:])
                    ot = out_pool.tile([128, V], f32, name="ot", tag="ot")
                    nc.vector.tensor_scalar_mul(ot[:], et[:], rec[:])
                    h0 = b * M + t * 512 + mc * 128
                    nc.sync.dma_start(out_d[h0:h0 + 128, par, :], ot[:])

        # software pipeline: stage3 of tile i runs on the PE right after the
        # conv matmuls of tile i+1 are queued, so DVE recombination of tile
        # i+1 overlaps PE streaming and rt(i) is long done when needed.
        tiles = [(b, t) for b in range(SEQ_PER_CORE) for t in range(NT)]
        prev = None
        for (b, t) in tiles:
            mid = (lambda p=prev: stage3_emit(*p)) if prev else None
            rt, pe_ts = conv_emit(b, t, mid=mid)
            prev = (b, t, rt, pe_ts)
        stage3_emit(*prev)

    nc.compile()
    return nc


def _get_nc():
    if "v2" not in _NC_CACHE:
        _NC_CACHE["v2"] = _build_nc()
    return _NC_CACHE["v2"]


_F8_GRID = None


def _f8_grid():
    global _F8_GRID
    if _F8_GRID is None:
        v = np.arange(256, dtype=np.uint8).view(F8).astype(np.float32)
        _F8_GRID = np.unique(v[np.isfinite(v)])
    return _F8_GRID


def _nearest2(x, grid):
    """Per-element (nearest, neighbor-on-the-other-side) fp8 grid values."""
    i = np.clip(np.searchsorted(grid, x), 1, len(grid) - 1)
    lo = grid[i - 1]
    hi = grid[i]
    pick_lo = (x - lo) <= (hi - x)
    return np.where(pick_lo, lo, hi), np.where(pick_lo, hi, lo)


def _joint_quant(a0, a1):
    """Sum-consistent fp8 quantization of (A0, A1, Am=A0+A1).

    The odd-path recombination Pm - P0 - P1 turns independent table
    rounding into 6-sigma^2 noise per tap pair (Am values are sqrt(2)
    larger, fp8 error is relative).  Choosing qm = rnd(a0+a1) and
    q1 = rnd(qm - q0) makes the even path see a single sum-rounding
    (2 s^2) and the odd path 4 s^2; a small candidate search polishes.
    """
    a0 = np.clip(a0, -240.0, 240.0).astype(np.float32)
    a1 = np.clip(a1, -240.0, 240.0).astype(np.float32)
    am = np.clip(a0 + a1, -240.0, 240.0).astype(np.float32)
    g = _f8_grid()
    q0c = _nearest2(a0, g)
    q1c = _nearest2(a1, g)
    bJ = bq0 = bq1 = bqm = None
    for q0 in q0c:
        e0 = q0 - a0
        for q1 in q1c:
            e1 = q1 - a1
            # odd-path optimum: em = (e0+e1)/2; try both fp8 neighbors
            for qm in _nearest2(np.clip(am + 0.5 * (e0 + e1),
                                        -240.0, 240.0), g):
                em = qm - am
                d0 = em - e0
                d1 = em - e1
                J = e0 * e0 + e1 * e1 + d0 * d0 + d1 * d1
                if bJ is None:
                    bJ, bq0, bq1, bqm = J, q0, q1, qm
                else:
                    m = J < bJ
                    bJ = np.where(m, J, bJ)
                    bq0 = np.where(m, q0, bq0)
                    bq1 = np.where(m, q1, bq1)
                    bqm = np.where(m, qm, bqm)
    return bq0, bq1, bqm


def _pack_tables(emb_table, conv_w, conv_b, lin_w, lin_b):
    """Host-side table precompute + fp8/fp16 packing (a weight repack)."""
    emb_table = np.asarray(emb_table, np.float32)
    conv_w = np.asarray(conv_w, np.float32)
    lin_w = np.asarray(lin_w, np.float32)
    conv_b = np.asarray(conv_b, np.float32)
    lin_b = np.asarray(lin_b, np.float32)
    # U[v,k,h] = sum_e emb[v,e] * conv_w[h,e,k]
    U = (emb_table @ conv_w.transpose(1, 0, 2).reshape(E, H * K))
    U = U.reshape(V, H, K).transpose(0, 2, 1)              # [V, K, H]
    G = U[:, ::-1, :]                                      # G_d = U_{15-d}
    A0 = np.ascontiguousarray(G[:, 0::2, :])               # [V, 8, H]
    A1 = np.ascontiguousarray(G[:, 1::2, :])
    # bias fold: A0[e=0] += b gives the even path +b via P0, and since
    # Am = A0f + A1 the 2-hot Pm picks the fold up twice while -P0 removes
    # one copy -> odd outputs also get exactly +b.
    A0[:, 0, :] += conv_b
    q0, q1, qm = _joint_quant(A0, A1)
    Astk = np.stack([q0, q1, qm])                          # [prod, V, 8, H]
    # ua[p, hc, prod, tap, vh, c] = Astk[prod, vh*128+p, tap, hc*128+c]
    ua = (Astk.reshape(3, 2, 128, 8, H8, 128)
          .transpose(2, 4, 0, 3, 1, 5))
    ua8 = np.ascontiguousarray(ua.astype(F8))

    pemb = emb_table @ lin_w[:, :E].T + lin_b[None, :]     # [V, V]
    W2T = lin_w[:, E:].T                                   # [H, V]
    w2 = np.ascontiguousarray(
        W2T.reshape(H8, 128, V).transpose(1, 0, 2).astype(F16))
    return ua8, w2, pemb


def _onehot3(tokens):
    """[128, 2, 3, B, MPAD] fp8 phase one-hots, left-padded with zeros."""
    tok = np.asarray(tokens).astype(np.int64)
    oh = np.zeros((128, 2, 3, B, MPAD), np.float32)
    b_idx = np.repeat(np.arange(B), M)
    col = np.tile(np.arange(M), B) + PADH
    for phase in range(2):
        t = tok[:, phase::2].ravel()
        oh[t % 128, t // 128, phase, b_idx, col] = 1.0
    oh[:, :, 2] = oh[:, :, 0] + oh[:, :, 1]
    return oh.astype(F8)


def kernel(input_sequence, emb_table, conv_w, conv_b, lin_w, lin_b):
    global LAST_RESULT
    import os
    if not TRACE:
        os.environ["BASS_NEVER_TRACE"] = "1"
    else:
        os.environ.pop("BASS_NEVER_TRACE", None)
    from concourse.bass_utils import run_bass_kernel_spmd

    ua8, w2, pemb = _pack_tables(emb_table, conv_w, conv_b, lin_w, lin_b)
    oh_full = _onehot3(input_sequence)

    tok = np.asarray(input_sequence).astype(np.int64)
    # phase-split emb-side logit rows: [B, par, M, V] -> per-unit packs
    rows = pemb[tok].astype(np.float32)                    # [B, S, V]
    rows = rows.reshape(B, M, 2, V).transpose(0, 2, 1, 3)  # [B, par, M, V]
    # [B, par, t, mc, p, v] -> unit (b, t, par), tile [p, mc, v]
    rows = rows.reshape(B, 2, NT, 4, 128, V).transpose(0, 2, 1, 4, 3, 5)
    rows = np.ascontiguousarray(rows.astype(F16))  # [B, NT, 2, 128, 4, V]

    in_maps = []
    for c in range(NCORES):
        b0 = c * SEQ_PER_CORE
        in_maps.append({
            "oh": np.ascontiguousarray(
                oh_full[:, :, :, b0:b0 + SEQ_PER_CORE, :]),
            "ua": ua8, "w2": w2,
            "pe": np.ascontiguousarray(
                rows[b0:b0 + SEQ_PER_CORE].reshape(-1, 128, 4, V)),
        })

    nc = _get_nc()
    res = run_bass_kernel_spmd(nc, in_maps, core_ids=list(range(NCORES)),
                               trace=TRACE)
    LAST_RESULT = res
    outs = [res.results[c]["out"].reshape(SEQ_PER_CORE * S, V)
            for c in range(NCORES)]
    full = np.concatenate(outs, axis=0).reshape(B, S, V)
    return np.ascontiguousarray(full.astype(np.float32))


# revision 25
# speedup vs baseline: 1.0147x; 1.0147x over previous
"""Trainium2 Bass kernel for CharPredictorMultirateFFN.

Model: emb = emb_table[tokens]; conv = relu(causal_conv1d(emb, K=16) + b);
logits = cat(emb, conv) @ lin_w.T + lin_b; out = softmax(logits).

Key algebraic restructure (tokens take only V=256 values):
  conv[s, h] = sum_d G_d[tok[s-d], h]   with G_d = U_{15-d},
  U[v,k,h] = sum_e emb[v,e] conv_w[h,e,k]
so the conv becomes 16 shifted one-hot matmuls with contract dim 256. The
one-hot is exact in fp8 and the tables are fp8 e4m3, so each tap is ONE
DoubleRow matmul (contract 256 in a single pass).

v2: polyphase KARATSUBA on the tap dimension. Split tokens into even/odd
phases (chi0[m]=tok[2m], chi1[m]=tok[2m+1]) and taps by delay parity
(A0_e = G_{2e}, A1_e = G_{2e+1}).  Then with half-rate 8-tap products
  P0 = A0 (*) chi0,  P1 = A1 (*) chi1,  Pm = (A0+A1) (*) (chi0+chi1)
the conv outputs are
  y_even[m] = P0[m] + P1[m-1]
  y_odd[m]  = Pm[m] - P0[m] - P1[m]
i.e. 24 half-rate tap-streams instead of 32 -> 25% less PE streaming time
(the kernel is purely PE-streaming-bound).  The recombination runs on the
otherwise-idle DVE/ACT engines, reading the product PSUM banks directly.
chi0+chi1 is a 2-hot vector (values {0,1,2}, exact in fp8).

Bias folds: conv_b into A0[e=0] (+b) and Am[e=0] (+b; the 2-hot hits the
fold twice, and -P0 removes one copy -> odd outputs get exactly +b).
lin_b is folded into the host-gathered emb-side logit rows.

Sharding: data-parallel over batch - 4 sequences per core on 8 cores, all
tables replicated, no collectives.
"""

import numpy as np
import ml_dtypes

B, S, V, E, H, K = 32, 2048, 256, 512, 1024, 16
NCORES = 8
SEQ_PER_CORE = B // NCORES            # 4
M = S // 2                            # 1024 half-positions per sequence
PADH = 8                              # left zero-pad (max tap back-reach)
MPAD = M + PADH + 8                   # 1040 (16-aligned)
NT = M // 512                         # 2 tiles of 512 half-positions per seq
H8 = H // 128                         # 8 h-chunks
F16 = np.float16
F8 = ml_dtypes.float8_e4m3fn

TRACE = False          # set True (e.g. from test.py) to capture NTFF profile
LAST_RESULT = None     # BassKernelResults of the most recent run

_NC_CACHE = {}


def _build_nc():
    """Build the Bass module (SPMD, identical program on every core)."""
    from contextlib import ExitStack
    import concourse.bacc as bacc
    import concourse.tile as tile
    import concourse.mybir as mybir

    f32 = mybir.dt.float32
    f16 = mybir.dt.float16
    f8 = mybir.dt.float8e4
    AF = mybir.ActivationFunctionType
    DR = mybir.MatmulPerfMode.DoubleRow

    nc = bacc.Bacc("TRN2", target_bir_lowering=False, debug=False,
                   num_devices=NCORES)

    # one-hot phase signals: [p, vh, phase(chi0,chi1,chim), seq, col]
    oh_d = nc.dram_tensor("oh", [128, 2, 3, SEQ_PER_CORE, MPAD], f8,
                          kind="ExternalInput").ap()
    # tables: [p, hc, prod(A0,A1,Am), tap, vh, c] - per-hc slabs contiguous
    ua_d = nc.dram_tensor("ua", [128, H8, 3, 8, 2, 128], f8,
                          kind="ExternalInput").ap()
    w2_d = nc.dram_tensor("w2", [128, H8, V], f16, kind="ExternalInput").ap()
    # host-gathered emb-half logit rows (lin_b folded in), phase-split:
    # [unit, p, mc, v] with unit = (b*NT + t)*2 + par, token halfpos
    # m = t*512 + mc*128 + p
    pe_d = nc.dram_tensor("pe", [SEQ_PER_CORE * NT * 2, 128, 4, V], f16,
                          kind="ExternalInput").ap()
    # out[halftok, par, v]; halftok = b*1024 + m; token = halftok*2+par
    out_d = nc.dram_tensor("out", [SEQ_PER_CORE * M, 2, V], f32,
                           kind="ExternalOutput").ap()

    with tile.TileContext(nc) as tc, ExitStack() as ctx:
        consts = ctx.enter_context(tc.tile_pool(name="consts", bufs=1))
        ua_t = consts.tile([128, H8, 3, 8, 2, 128], f8, name="ua_t")
        oh_t = consts.tile([128, 2, 3, SEQ_PER_CORE, MPAD], f8, name="oh_t")
        w2_t = consts.tile([128, H8, V], f16, name="w2_t")
        carry_t = consts.tile([128, H8, 1], f16, name="carry_t")

        # staggered loads ordered along the critical path: the first conv
        # unit consumes ua[:, hc=0, prod] in prod order against
        # oh[phase, seq=0, cols<528], so stream those first - per-prod
        # table slabs (256KB) on sync, per-phase oh heads (135KB) on the
        # Activation HWDGE queue - and backfill the rest afterwards.
        # per-(hc, prod) slabs for hc0-2 split across both HWDGE queues in
        # exact consumption order (the first tile has no stage3 interleave,
        # so its table demand rate is the highest of the whole kernel)
        nc.scalar.dma_start(oh_t[:, :, 0, 0, 0:528], oh_d[:, :, 0, 0, 0:528])
        nc.sync.dma_start(ua_t[:, 0, 0], ua_d[:, 0, 0])
        nc.scalar.dma_start(oh_t[:, :, 1, 0, 0:528], oh_d[:, :, 1, 0, 0:528])
        nc.sync.dma_start(ua_t[:, 0, 1], ua_d[:, 0, 1])
        nc.scalar.dma_start(oh_t[:, :, 2, 0, 0:528], oh_d[:, :, 2, 0, 0:528])
        nc.scalar.dma_start(ua_t[:, 0, 2], ua_d[:, 0, 2])
        nc.sync.dma_start(ua_t[:, 1, 0], ua_d[:, 1, 0])
        nc.sync.dma_start(ua_t[:, 1, 1], ua_d[:, 1, 1])
        nc.sync.dma_start(ua_t[:, 1, 2], ua_d[:, 1, 2])
        nc.scalar.dma_start(ua_t[:, 2, 0], ua_d[:, 2, 0])
        nc.sync.dma_start(ua_t[:, 2, 1], ua_d[:, 2, 1])
        nc.sync.dma_start(ua_t[:, 2, 2], ua_d[:, 2, 2])
        nc.scalar.dma_start(oh_t[:, :, :, 0, 528:MPAD],
                            oh_d[:, :, :, 0, 528:MPAD])
        for hc in range(3, H8):
            nc.sync.dma_start(ua_t[:, hc], ua_d[:, hc])
        for b in range(1, SEQ_PER_CORE):
            nc.scalar.dma_start(oh_t[:, :, :, b, :], oh_d[:, :, :, b, :])
        nc.sync.dma_start(w2_t[:], w2_d[:])

        pe_pool = ctx.enter_context(tc.tile_pool(name="pep", bufs=4))
        warm_pool = ctx.enter_context(tc.tile_pool(name="warm", bufs=1))
        rt_pool = ctx.enter_context(tc.tile_pool(name="rtp", bufs=2))
        s1_pool = ctx.enter_context(tc.tile_pool(name="s1p", bufs=3))
        t1_pool = ctx.enter_context(tc.tile_pool(name="t1p", bufs=3))
        cps = ctx.enter_context(tc.tile_pool(name="cps", bufs=6, space="PSUM"))
        lps = ctx.enter_context(tc.tile_pool(name="lps", bufs=2, space="PSUM"))
        sm_pool = ctx.enter_context(tc.tile_pool(name="smp", bufs=4))
        out_pool = ctx.enter_context(tc.tile_pool(name="outp", bufs=4))

        # PE warm-up: the HAM clock gate keeps the PE at 1.2 GHz until it
        # has seen ~3.4us of sustained activity.  The first real matmul
        # can't start until its tables land (~10us of DMA), so burn the
        # wait on dependency-free dummy matmuls over a memset scratch tile
        # and a throwaway PSUM bank - the real matmuls then start warm.
        wsc = warm_pool.tile([128, 2, 256], f8, name="wsc")
        nc.vector.memset(wsc[:], 0.0)
        wps = lps.tile([128, 256], f32, name="wps", tag="psl")
        for i in range(14):
            nc.tensor.matmul(wps[:], wsc[:, :, 0:128], wsc[:],
                             start=(i == 0), stop=(i == 13),
                             perf_mode=DR)

        def conv_emit(b, t, mid=None):
            """Karatsuba conv for 512 half-positions (1024 tokens).

            Returns rt tile [128, 2(par), H8, 512] f16 = relu(conv) with
            h on partitions, and the pe tiles for both parities.
            """
            pe_ts = []
            for par in range(2):
                pe_t = pe_pool.tile([128, 4, V], f16, name="pe_t", tag="pe")
                nc.sync.dma_start(pe_t[:], pe_d[(b * NT + t) * 2 + par])
                pe_ts.append(pe_t)
            rt = rt_pool.tile([128, 2, H8, 512], f16, name="rt", tag="rt")
            c0 = t * 512 + PADH
            for hc in range(H8):
                bp0 = cps.tile([128, 512], f32, name="bp0", tag="cp")
                bp1 = cps.tile([128, 512], f32, name="bp1", tag="cp")
                bpm = cps.tile([128, 512], f32, name="bpm", tag="cp")
                for prod, bank in ((0, bp0), (1, bp1), (2, bpm)):
                    for e in range(8):
                        nc.tensor.matmul(
                            bank[:],
                            ua_t[:, hc, prod, e],
                            oh_t[:, :, prod if prod < 2 else 2, b,
                                 c0 - e: c0 - e + 512],
                            start=(e == 0), stop=(e == 7),
                            perf_mode=DR)
                # S1[j] = P1[m0-1+j], j=0..512  (513 wide)
                s1 = s1_pool.tile([128, 513], f16, name="s1", tag="s1")
                if t == 0:
                    nc.vector.memset(s1[:, 0:1], 0.0)
                else:
                    nc.vector.tensor_copy(s1[:, 0:1], carry_t[:, hc, :])
                nc.scalar.copy(s1[:, 1:513], bp1[:])
                nc.scalar.copy(carry_t[:, hc, :], bp1[:, 511:512])
                # even: relu(P0[m] + P1[m-1])
                nc.vector.tensor_add(rt[:, 0, hc, :], bp0[:], s1[:, 0:512])
                nc.vector.tensor_scalar_max(rt[:, 0, hc, :],
                                            rt[:, 0, hc, :], 0.0)
                # odd: relu(Pm[m] - P1[m] - P0[m])
                t1 = t1_pool.tile([128, 512], f16, name="t1", tag="t1")
                nc.vector.tensor_sub(t1[:], bpm[:], s1[:, 1:513])
                nc.vector.tensor_sub(rt[:, 1, hc, :], t1[:], bp0[:])
                nc.vector.tensor_scalar_max(rt[:, 1, hc, :],
                                            rt[:, 1, hc, :], 0.0)
                if hc == 1 and mid is not None:
                    # interleave the previous tile's stage3 here: its PE
                    # matmuls need no conv PSUM banks, so they cover the
                    # tile-boundary window while DVE drains this tile's
                    # first combos and frees banks for hc2+
                    mid()
            return rt, pe_ts

        def stage3_emit(b, t, rt, pe_ts):
            """logits = rt@W2T (PE) + gathered emb rows (DVE), softmax."""
            for par in range(2):
                pe_t = pe_ts[par]
                for mc in range(4):
                    psl = lps.tile([128, V], f32, name="psl", tag="psl")
                    for h8 in range(H8):
                        nc.tensor.matmul(
                            psl[:], rt[:, par, h8, mc * 128:(mc + 1) * 128],
                            w2_t[:, h8, :],
                            start=(h8 == 0), stop=(h8 == H8 - 1))
                    li = sm_pool.tile([128, V], f32, name="li", tag="li")
                    nc.vector.tensor_add(li[:], psl[:], pe_t[:, mc, :])
                    et = sm_pool.tile([128, V], f32, name="et", tag="et")
                    ssum = sm_pool.tile([128, 1], f32, name="ssum", tag="ss")
                    nc.scalar.activation(et[:], li[:], AF.Exp,
                                         accum_out=ssum[:])
                    rec = sm_pool.tile([128, 1], f32, name="rec", tag="rec")
                    nc.vector.reciprocal(rec[:], ssum[:])
                    ot = out_pool.tile([128, V], f32, name="ot", tag="ot")
                    nc.vector.tensor_scalar_mul(ot[:], et[:], rec[:])
                    h0 = b * M + t * 512 + mc * 128
                    nc.sync.dma_start(out_d[h0:h0 + 128, par, :], ot[:])

        # software pipeline: stage3 of tile i runs on the PE right after the
        # conv matmuls of tile i+1 are queued, so DVE recombination of tile
        # i+1 overlaps PE streaming and rt(i) is long done when needed.
        tiles = [(b, t) for b in range(SEQ_PER_CORE) for t in range(NT)]
        prev = None
        for (b, t) in tiles:
            mid = (lambda p=prev: stage3_emit(*p)) if prev else None
            rt, pe_ts = conv_emit(b, t, mid=mid)
            prev = (b, t, rt, pe_ts)
        stage3_emit(*prev)

    nc.compile()
    return nc


def _get_nc():
    if "v2" not in _NC_CACHE:
        _NC_CACHE["v2"] = _build_nc()
    return _NC_CACHE["v2"]


_F8_GRID = None


def _f8_grid():
    global _F8_GRID
    if _F8_GRID is None:
        v = np.arange(256, dtype=np.uint8).view(F8).astype(np.float32)
        _F8_GRID = np.unique(v[np.isfinite(v)])
    return _F8_GRID


def _nearest2(x, grid):
    """Per-element (nearest, neighbor-on-the-other-side) fp8 grid values."""
    i = np.clip(np.searchsorted(grid, x), 1, len(grid) - 1)
    lo = grid[i - 1]
    hi = grid[i]
    pick_lo = (x - lo) <= (hi - x)
    return np.where(pick_lo, lo, hi), np.where(pick_lo, hi, lo)


def _joint_quant(a0, a1):
    """Sum-consistent fp8 quantization of (A0, A1, Am=A0+A1).

    The odd-path recombination Pm - P0 - P1 turns independent table
    rounding into 6-sigma^2 noise per tap pair (Am values are sqrt(2)
    larger, fp8 error is relative).  Choosing qm = rnd(a0+a1) and
    q1 = rnd(qm - q0) makes the even path see a single sum-rounding
    (2 s^2) and the odd path 4 s^2; a small candidate search polishes.
    """
    a0 = np.clip(a0, -240.0, 240.0).astype(np.float32)
    a1 = np.clip(a1, -240.0, 240.0).astype(np.float32)
    am = np.clip(a0 + a1, -240.0, 240.0).astype(np.float32)
    g = _f8_grid()
    q0c = _nearest2(a0, g)
    q1c = _nearest2(a1, g)
    bJ = bq0 = bq1 = bqm = None
    for q0 in q0c:
        e0 = q0 - a0
        for q1 in q1c:
            e1 = q1 - a1
            # odd-path optimum: em = (e0+e1)/2; try both fp8 neighbors
            for qm in _nearest2(np.clip(am + 0.5 * (e0 + e1),
                                        -240.0, 240.0), g):
                em = qm - am
                d0 = em - e0
                d1 = em - e1
                J = e0 * e0 + e1 * e1 + d0 * d0 + d1 * d1
                if bJ is None:
                    bJ, bq0, bq1, bqm = J, q0, q1, qm
                else:
                    m = J < bJ
                    bJ = np.where(m, J, bJ)
                    bq0 = np.where(m, q0, bq0)
                    bq1 = np.where(m, q1, bq1)
                    bqm = np.where(m, qm, bqm)
    return bq0, bq1, bqm


def _pack_tables(emb_table, conv_w, conv_b, lin_w, lin_b):
    """Host-side table precompute + fp8/fp16 packing (a weight repack)."""
    emb_table = np.asarray(emb_table, np.float32)
    conv_w = np.asarray(conv_w, np.float32)
    lin_w = np.asarray(lin_w, np.float32)
    conv_b = np.asarray(conv_b, np.float32)
    lin_b = np.asarray(lin_b, np.float32)
    # U[v,k,h] = sum_e emb[v,e] * conv_w[h,e,k]
    U = (emb_table @ conv_w.transpose(1, 0, 2).reshape(E, H * K))
    U = U.reshape(V, H, K).transpose(0, 2, 1)              # [V, K, H]
    G = U[:, ::-1, :]                                      # G_d = U_{15-d}
    A0 = np.ascontiguousarray(G[:, 0::2, :])               # [V, 8, H]
    A1 = np.ascontiguousarray(G[:, 1::2, :])
    # bias fold: A0[e=0] += b gives the even path +b via P0, and since
    # Am = A0f + A1 the 2-hot Pm picks the fold up twice while -P0 removes
    # one copy -> odd outputs also get exactly +b.
    A0[:, 0, :] += conv_b
    q0, q1, qm = _joint_quant(A0, A1)
    Astk = np.stack([q0, q1, qm])                          # [prod, V, 8, H]
    # ua[p, hc, prod, tap, vh, c] = Astk[prod, vh*128+p, tap, hc*128+c]
    ua = (Astk.reshape(3, 2, 128, 8, H8, 128)
          .transpose(2, 4, 0, 3, 1, 5))
    ua8 = np.ascontiguousarray(ua.astype(F8))

    pemb = emb_table @ lin_w[:, :E].T + lin_b[None, :]     # [V, V]
    W2T = lin_w[:, E:].T                                   # [H, V]
    w2 = np.ascontiguousarray(
        W2T.reshape(H8, 128, V).transpose(1, 0, 2).astype(F16))
    return ua8, w2, pemb


def _onehot3(tokens):
    """[128, 2, 3, B, MPAD] fp8 phase one-hots, left-padded with zeros."""
    tok = np.asarray(tokens).astype(np.int64)
    oh = np.zeros((128, 2, 3, B, MPAD), np.float32)
    b_idx = np.repeat(np.arange(B), M)
    col = np.tile(np.arange(M), B) + PADH
    for phase in range(2):
        t = tok[:, phase::2].ravel()
        oh[t % 128, t // 128, phase, b_idx, col] = 1.0
    oh[:, :, 2] = oh[:, :, 0] + oh[:, :, 1]
    return oh.astype(F8)


def kernel(input_sequence, emb_table, conv_w, conv_b, lin_w, lin_b):
    global LAST_RESULT
    import os
    if not TRACE:
        os.environ["BASS_NEVER_TRACE"] = "1"
    else:
        os.environ.pop("BASS_NEVER_TRACE", None)
    from concourse.bass_utils import run_bass_kernel_spmd

    ua8, w2, pemb = _pack_tables(emb_table, conv_w, conv_b, lin_w, lin_b)
    oh_full = _onehot3(input_sequence)

    tok = np.asarray(input_sequence).astype(np.int64)
    # phase-split emb-side logit rows: [B, par, M, V] -> per-unit packs
    rows = pemb[tok].astype(np.float32)                    # [B, S, V]
    rows = rows.reshape(B, M, 2, V).transpose(0, 2, 1, 3)  # [B, par, M, V]
    # [B, par, t, mc, p, v] -> unit (b, t, par), tile [p, mc, v]
    rows = rows.reshape(B, 2, NT, 4, 128, V).transpose(0, 2, 1, 4, 3, 5)
    rows = np.ascontiguousarray(rows.astype(F16))  # [B, NT, 2, 128, 4, V]

    in_maps = []
    for c in range(NCORES):
        b0 = c * SEQ_PER_CORE
        in_maps.append({
            "oh": np.ascontiguousarray(
                oh_full[:, :, :, b0:b0 + SEQ_PER_CORE, :]),
            "ua": ua8, "w2": w2,
            "pe": np.ascontiguousarray(
                rows[b0:b0 + SEQ_PER_CORE].reshape(-1, 128, 4, V)),
        })

    nc = _get_nc()
    res = run_bass_kernel_spmd(nc, in_maps, core_ids=list(range(NCORES)),
                               trace=TRACE)
    LAST_RESULT = res
    outs = [res.results[c]["out"].reshape(SEQ_PER_CORE * S, V)
            for c in range(NCORES)]
    full = np.concatenate(outs, axis=0).reshape(B, S, V)
    return np.ascontiguousarray(full.astype(np.float32))
